# revision 15
# baseline (speedup 1.0000x reference)
"""Direct Conv2d (NCHW, OIHW, VALID, stride 1) on 8 Trainium2 NeuronCores.

Problem: input [16, 4, 512, 512] f32, filter [8, 4, 3, 3] f32
         -> output [16, 8, 510, 510] f32.

Sharding: data-parallel over batch N: 2 images per core, filter replicated.

The kernel is limited by three near-equal walls, all sized deliberately:
  - HBM traffic: all device I/O is bf16 (host downcasts input, device
    stores bf16 output, host upcasts). 13.2 MB/core vs 26 MB at f32.
    Measured rel-err of the full bf16 pipeline is ~4e-3 (tolerance 2e-2);
    products are bf16*bf16 but accumulation stays f32 in PSUM.
  - PE columns: 6 matmuls of N=510 per 30-row supertile (102 cyc/row).
    bf16 matmuls issue at 215 ns warm (measured) with LDWEIGHTS hidden,
    and the 6-MM bursts keep the PE's HAM clock-gate at 2.4 GHz.
  - DMA descriptor generation (~6 ns/chunk): the row-pair-interleaved
    output layout makes store chunks 2040 B (2 consecutive output rows
    per partition), halving store descriptor count vs a naive layout.

Per-core algorithm (all shapes hardcoded):
  Output rows are processed in supertiles of 30 rows = 15 row-pairs;
  510 = 17 x 30 exactly, so there is no tail tile. Sub-block b in {0,1}
  computes rows h0 + 2j + b for j in [0,15) as 3 accumulating bf16
  matmuls (one per filter column s, a free-dim offset into the shared
  input tile):

    psum_b[(j,m), w] += sum_{q,c} Wb_s[(q,c), (j,m)] * in[c, h0+q, w+s]

  with Wb_s[(q,c), (j,m)] = filter[m, c, q-2j-b, s] for 0 <= q-2j-b < 3
  (banded matrices built host-side). K = 32 input rows x 4 channels =
  128, M = 15 j-pairs x 8 out-channels = 120, N = 510.

  The input tile [128, 512] (partition = q*4+c) is one SWDGE (gpsimd)
  DMA per supertile; the last supertile's rows 480..511 exactly use up
  the image. The two PSUM results are cast-copied f32->bf16 into one
  SBUF tile [120, 1020] (DVE for b=0, ACT for b=1 - they run in
  parallel), where partition (j,m) holds output rows h0+2j and h0+2j+1
  back to back, making the store's HBM chunks 2040 B contiguous. Stores
  go through the sync HWDGE ring; its DRAM AP leads with the 15-wide j
  dim (15 SDMA engines).
"""

import os

os.environ.setdefault("MYCRO_LOCAL_CACHE", "1")

import numpy as np
import ml_dtypes

import concourse.bacc as bacc
import concourse.mybir as mybir
import concourse.tile as tile
from concourse.bass_utils import run_bass_kernel_spmd

N_CORES = 8
IMG_PER_CORE = 2
C_IN, H, W = 4, 512, 512
C_OUT, R, S = 8, 3, 3
HO, WO = 510, 510

JP = 15               # row-pairs per supertile
SUPER = 2 * JP        # 30 output rows per supertile
QB = SUPER + 2        # 32 input rows per supertile
MDIM = C_OUT * JP     # 120 matmul output partitions
NSUPER = HO // SUPER  # 17, exact

DT = mybir.dt.bfloat16

# Set by test harness: TRACE=True -> capture NTFF profile, LAST_EXEC_NS set.
TRACE = False
TRACE_DIR = None
LAST_EXEC_NS = None
LAST_RESULTS = None

_NC_CACHE = {}


def build_wT(filt: np.ndarray) -> np.ndarray:
    """Banded weight matrices [128, 6*120] bf16 from filter [8, 4, 3, 3].

    Column block (b*3+s)*120: Wb_s[q*4+c, j*8+m] = filt[m, c, q-2j-b, s]
    for 0 <= q-2j-b < 3 (else 0), q in [0,32), j in [0,15).
    """
    wt = np.zeros((128, 6 * MDIM), np.float32)
    q = np.arange(QB)
    j = np.arange(JP)
    for b in range(2):
        r = q[:, None] - 2 * j[None, :] - b          # [32, 15]
        valid = (r >= 0) & (r < R)
        qi, ji = np.nonzero(valid)
        for s in range(S):
            # filt[m, c, r, s] -> wt[q*4+c, (b*3+s)*120 + j*8+m]
            blk = (b * 3 + s) * MDIM
            for c in range(C_IN):
                wt[qi * 4 + c, blk + ji * 8 + np.arange(C_OUT)[:, None]] = filt[
                    :, c, r[qi, ji], s
                ]
    return wt.astype(ml_dtypes.bfloat16)


def conv_body(tc, y, x, wt_d):
    nc = tc.nc
    with (
        tc.tile_pool(name="wt", bufs=1) as wt_pool,
        tc.tile_pool(name="xt", bufs=10) as x_pool,
        tc.tile_pool(name="yt", bufs=8) as y_pool,
        tc.tile_pool(name="ps", bufs=8, space="PSUM") as ps_pool,
    ):
        wt = wt_pool.tile([128, 6 * MDIM], DT)
        # weights on the sync HWDGE ring: it is idle until the first store,
        # while the scalar ring starts with a 1.3 us ACT_TABLE_LOAD.
        nc.sync.dma_start(out=wt[:, :], in_=wt_d[:, :])
        # PE warmup: 8 dummy matmuls gated only on a DVE memset run during
        # the first input-load + weight-load latency (~3.5 us) and lift the
        # PE's HAM clock gate to 2.4 GHz (needs ~3.4 us of sustained PE
        # activity) just as the first real matmul becomes ready; without
        # them the first ~3.4 us of real matmuls run at 1.2 GHz.
        wu_w = x_pool.tile([128, W], DT, name="xt")
        wu_ps = ps_pool.tile([MDIM, WO], mybir.dt.float32, name="ps")
        nc.vector.memset(wu_w[:, :], 0.0)
        for _ in range(8):
            nc.tensor.matmul(
                wu_ps[:, :],
                lhsT=wu_w[:, 0:MDIM],
                rhs=wu_w[:, 0:WO],
                start=True,
                stop=True,
            )
        for i in range(IMG_PER_CORE):
            for t in range(NSUPER):
                h0 = t * SUPER
                xt = x_pool.tile([128, W], DT, name="xt")
                # partition (q*4+c) <- in[c, h0+q, :]; SWDGE so descriptor
                # generation runs on gpsimd, off the store ring.
                nc.gpsimd.dma_start(
                    out=xt[:, :],
                    in_=x[i, :, h0 : h0 + QB, :].transpose([1, 0, 2]),
                )
                yt = y_pool.tile([MDIM, 2 * WO], DT)
                for b in range(2):
                    ps = ps_pool.tile([MDIM, WO], mybir.dt.float32, name="ps")
                    for s in range(S):
                        nc.tensor.matmul(
                            ps[:, :],
                            lhsT=wt[:, (b * 3 + s) * MDIM : (b * 3 + s + 1) * MDIM],
                            rhs=xt[:, s : s + WO],
                            start=(s == 0),
                            stop=(s == S - 1),
                        )
                    # PSUM f32 -> SBUF bf16 cast; partition (j,m) collects
                    # rows h0+2j (cols 0:510) and h0+2j+1 (cols 510:1020).
                    if b == 0:
                        nc.vector.tensor_copy(yt[:, b * WO : (b + 1) * WO], ps[:, :])
                    else:
                        nc.scalar.copy(yt[:, b * WO : (b + 1) * WO], ps[:, :])
                # dst element (m, 2j+b, w) <- src partition (j*8+m), free
                # (b*510+w): 2040 B contiguous chunks, 15-wide outer dim.
                if i == IMG_PER_CORE - 1 and t == NSUPER - 1:
                    # Final supertile: store each column half as soon as its
                    # PSUM copy lands, so the b=0 half drains while the b=1
                    # matmuls/copy still run - shortens the kernel tail.
                    dstb = y[i, :, h0 : h0 + SUPER, :].rearrange(
                        "m (j b) w -> b j m w", b=2
                    )
                    nc.sync.dma_start(out=dstb[0], in_=yt[:, 0:WO])
                    nc.sync.dma_start(out=dstb[1], in_=yt[:, WO : 2 * WO])
                else:
                    dst = y[i, :, h0 : h0 + SUPER, :].rearrange(
                        "m (j b) w -> j m (b w)", b=2
                    )
                    nc.sync.dma_start(out=dst, in_=yt[:, :])


def build_nc(enable_asserts: bool = False):
    nc = bacc.Bacc(
        "TRN2",
        target_bir_lowering=False,
        debug=False,
        enable_asserts=enable_asserts,
        num_devices=N_CORES,
    )
    x = nc.dram_tensor("x", [IMG_PER_CORE, C_IN, H, W], DT, kind="ExternalInput").ap()
    wt_d = nc.dram_tensor("wt", [128, 6 * MDIM], DT, kind="ExternalInput").ap()
    y = nc.dram_tensor(
        "y", [IMG_PER_CORE, C_OUT, HO, WO], DT, kind="ExternalOutput"
    ).ap()
    with tile.TileContext(nc) as tc:
        conv_body(tc, y, x, wt_d)
    nc.compile()
    return nc


def kernel(_input: np.ndarray, _filter: np.ndarray) -> np.ndarray:
    global LAST_EXEC_NS, LAST_RESULTS
    _input = np.asarray(_input, dtype=np.float32).astype(ml_dtypes.bfloat16)
    _filter = np.asarray(_filter, dtype=np.float32)

    key = DT
    if key not in _NC_CACHE:
        _NC_CACHE[key] = build_nc()
    nc = _NC_CACHE[key]

    wT = build_wT(_filter)
    in_maps = [
        {
            "x": np.ascontiguousarray(_input[IMG_PER_CORE * i : IMG_PER_CORE * (i + 1)]),
            "wt": wT,
        }
        for i in range(N_CORES)
    ]
    res = run_bass_kernel_spmd(
        nc, in_maps, list(range(N_CORES)), trace=TRACE, tmpdir=TRACE_DIR
    )
    LAST_EXEC_NS = res.exec_time_ns
    LAST_RESULTS = res
    out = np.concatenate([np.asarray(r["y"]) for r in res.results], axis=0)
    return out.astype(np.float32)


# revision 16
# speedup vs baseline: 1.0248x; 1.0248x over previous
"""Direct Conv2d (NCHW, OIHW, VALID, stride 1) on 8 Trainium2 NeuronCores.

Problem: input [16, 4, 512, 512] f32, filter [8, 4, 3, 3] f32
         -> output [16, 8, 510, 510] f32.

Sharding: data-parallel over batch N: 2 images per core, filter replicated.

The kernel is limited by three near-equal walls, all sized deliberately:
  - HBM traffic: all device I/O is bf16 (host downcasts input, device
    stores bf16 output, host upcasts). 13.2 MB/core vs 26 MB at f32.
    Measured rel-err of the full bf16 pipeline is ~4e-3 (tolerance 2e-2);
    products are bf16*bf16 but accumulation stays f32 in PSUM.
  - PE columns: 6 matmuls of N=510 per 30-row supertile (102 cyc/row).
    bf16 matmuls issue at 215 ns warm (measured) with LDWEIGHTS hidden,
    and the 6-MM bursts keep the PE's HAM clock-gate at 2.4 GHz.
  - DMA descriptor generation (~6 ns/chunk): the row-pair-interleaved
    output layout makes store chunks 2040 B (2 consecutive output rows
    per partition), halving store descriptor count vs a naive layout.

Per-core algorithm (all shapes hardcoded):
  Output rows are processed in supertiles of 30 rows = 15 row-pairs;
  510 = 17 x 30 exactly, so there is no tail tile. Sub-block b in {0,1}
  computes rows h0 + 2j + b for j in [0,15) as 3 accumulating bf16
  matmuls (one per filter column s, a free-dim offset into the shared
  input tile):

    psum_b[(j,m), w] += sum_{q,c} Wb_s[(q,c), (j,m)] * in[c, h0+q, w+s]

  with Wb_s[(q,c), (j,m)] = filter[m, c, q-2j-b, s] for 0 <= q-2j-b < 3
  (banded matrices built host-side). K = 32 input rows x 4 channels =
  128, M = 15 j-pairs x 8 out-channels = 120, N = 510.

  The input tile [128, 512] (partition = q*4+c) is one SWDGE (gpsimd)
  DMA per supertile; the last supertile's rows 480..511 exactly use up
  the image. The two PSUM results are cast-copied f32->bf16 into one
  SBUF tile [120, 1020] (DVE for b=0, ACT for b=1 - they run in
  parallel), where partition (j,m) holds output rows h0+2j and h0+2j+1
  back to back, making the store's HBM chunks 2040 B contiguous. Stores
  go through the sync HWDGE ring; its DRAM AP leads with the 15-wide j
  dim (15 SDMA engines).
"""

import os

os.environ.setdefault("MYCRO_LOCAL_CACHE", "1")

import numpy as np
import ml_dtypes

import concourse.bacc as bacc
import concourse.mybir as mybir
import concourse.tile as tile
from concourse.bass_utils import run_bass_kernel_spmd

N_CORES = 8
IMG_PER_CORE = 2
C_IN, H, W = 4, 512, 512
C_OUT, R, S = 8, 3, 3
HO, WO = 510, 510

JP = 15               # row-pairs per supertile
SUPER = 2 * JP        # 30 output rows per supertile
QB = SUPER + 2        # 32 input rows per supertile
MDIM = C_OUT * JP     # 120 matmul output partitions
NSUPER = HO // SUPER  # 17, exact

DT = mybir.dt.bfloat16

# Set by test harness: TRACE=True -> capture NTFF profile, LAST_EXEC_NS set.
TRACE = False
TRACE_DIR = None
LAST_EXEC_NS = None
LAST_RESULTS = None

_NC_CACHE = {}


def build_wT(filt: np.ndarray) -> np.ndarray:
    """Banded weight matrices [128, 6*120] bf16 from filter [8, 4, 3, 3].

    Column block (b*3+s)*120: Wb_s[q*4+c, j*8+m] = filt[m, c, q-2j-b, s]
    for 0 <= q-2j-b < 3 (else 0), q in [0,32), j in [0,15).
    """
    wt = np.zeros((128, 6 * MDIM), np.float32)
    q = np.arange(QB)
    j = np.arange(JP)
    for b in range(2):
        r = q[:, None] - 2 * j[None, :] - b          # [32, 15]
        valid = (r >= 0) & (r < R)
        qi, ji = np.nonzero(valid)
        for s in range(S):
            # filt[m, c, r, s] -> wt[q*4+c, (b*3+s)*120 + j*8+m]
            blk = (b * 3 + s) * MDIM
            for c in range(C_IN):
                wt[qi * 4 + c, blk + ji * 8 + np.arange(C_OUT)[:, None]] = filt[
                    :, c, r[qi, ji], s
                ]
    return wt.astype(ml_dtypes.bfloat16)


def conv_body(tc, y, x, wt_d):
    nc = tc.nc
    with (
        tc.tile_pool(name="wt", bufs=1) as wt_pool,
        tc.tile_pool(name="xt", bufs=10) as x_pool,
        tc.tile_pool(name="yt", bufs=8) as y_pool,
        tc.tile_pool(name="ps", bufs=8, space="PSUM") as ps_pool,
    ):
        wt = wt_pool.tile([128, 6 * MDIM], DT)
        # weights on the sync HWDGE ring: it is idle until the first store,
        # while the scalar ring starts with a 1.3 us ACT_TABLE_LOAD.
        nc.sync.dma_start(out=wt[:, :], in_=wt_d[:, :])
        # PE warmup: 8 dummy matmuls gated only on a DVE memset run during
        # the first input-load + weight-load latency (~3.5 us) and lift the
        # PE's HAM clock gate to 2.4 GHz (needs ~3.4 us of sustained PE
        # activity) just as the first real matmul becomes ready; without
        # them the first ~3.4 us of real matmuls run at 1.2 GHz.
        wu_w = x_pool.tile([128, W], DT, name="xt")
        wu_ps = ps_pool.tile([MDIM, WO], mybir.dt.float32, name="ps")
        nc.vector.memset(wu_w[:, :], 0.0)
        for _ in range(8):
            nc.tensor.matmul(
                wu_ps[:, :],
                lhsT=wu_w[:, 0:MDIM],
                rhs=wu_w[:, 0:WO],
                start=True,
                stop=True,
            )
        for i in range(IMG_PER_CORE):
            for t in range(NSUPER):
                h0 = t * SUPER
                xt = x_pool.tile([128, W], DT, name="xt")
                # partition (q*4+c) <- in[c, h0+q, :]; SWDGE so descriptor
                # generation runs on gpsimd, off the store ring.
                nc.gpsimd.dma_start(
                    out=xt[:, :],
                    in_=x[i, :, h0 : h0 + QB, :].transpose([1, 0, 2]),
                )
                yt = y_pool.tile([MDIM, 2 * WO], DT)
                for b in range(2):
                    ps = ps_pool.tile([MDIM, WO], mybir.dt.float32, name="ps")
                    for s in range(S):
                        nc.tensor.matmul(
                            ps[:, :],
                            lhsT=wt[:, (b * 3 + s) * MDIM : (b * 3 + s + 1) * MDIM],
                            rhs=xt[:, s : s + WO],
                            start=(s == 0),
                            stop=(s == S - 1),
                        )
                    # PSUM f32 -> SBUF bf16 cast; partition (j,m) collects
                    # rows h0+2j (cols 0:510) and h0+2j+1 (cols 510:1020).
                    if b == 0:
                        nc.vector.tensor_copy(yt[:, b * WO : (b + 1) * WO], ps[:, :])
                    else:
                        nc.scalar.copy(yt[:, b * WO : (b + 1) * WO], ps[:, :])
                # dst element (m, 2j+b, w) <- src partition (j*8+m), free
                # (b*510+w): 2040 B contiguous chunks, 15-wide outer dim.
                dst = y[i, :, h0 : h0 + SUPER, :].rearrange(
                    "m (j b) w -> j m (b w)", b=2
                )
                nc.sync.dma_start(out=dst, in_=yt[:, :])


def build_nc(enable_asserts: bool = False):
    nc = bacc.Bacc(
        "TRN2",
        target_bir_lowering=False,
        debug=False,
        enable_asserts=enable_asserts,
        num_devices=N_CORES,
    )
    x = nc.dram_tensor("x", [IMG_PER_CORE, C_IN, H, W], DT, kind="ExternalInput").ap()
    wt_d = nc.dram_tensor("wt", [128, 6 * MDIM], DT, kind="ExternalInput").ap()
    y = nc.dram_tensor(
        "y", [IMG_PER_CORE, C_OUT, HO, WO], DT, kind="ExternalOutput"
    ).ap()
    with tile.TileContext(nc) as tc:
        conv_body(tc, y, x, wt_d)
    nc.compile()
    return nc


def kernel(_input: np.ndarray, _filter: np.ndarray) -> np.ndarray:
    global LAST_EXEC_NS, LAST_RESULTS
    _input = np.asarray(_input, dtype=np.float32).astype(ml_dtypes.bfloat16)
    _filter = np.asarray(_filter, dtype=np.float32)

    key = DT
    if key not in _NC_CACHE:
        _NC_CACHE[key] = build_nc()
    nc = _NC_CACHE[key]

    wT = build_wT(_filter)
    in_maps = [
        {
            "x": np.ascontiguousarray(_input[IMG_PER_CORE * i : IMG_PER_CORE * (i + 1)]),
            "wt": wT,
        }
        for i in range(N_CORES)
    ]
    res = run_bass_kernel_spmd(
        nc, in_maps, list(range(N_CORES)), trace=TRACE, tmpdir=TRACE_DIR
    )
    LAST_EXEC_NS = res.exec_time_ns
    LAST_RESULTS = res
    out = np.concatenate([np.asarray(r["y"]) for r in res.results], axis=0)
    return out.astype(np.float32)
